# revision 30
# baseline (speedup 1.0000x reference)
"""Causal self-attention kernel v8 for 8 Trainium2 NeuronCores (Bass/Tile).

B=4, T=2048, C=1024, 16 heads. 8 cores = 4 batches x 2 head-groups (8 heads
each); host sums the two projection partials per batch.

Design (vs the 330913ns v3 baseline):
- v computed pre-transposed (lhsT = x^T chunks, rhs = w_v) -> kills the 64
  PE transposes + extra copies; v psum lands directly in [keys, d] layout
- single fused schedule: qkv tile matmuls for later head-pairs are emitted
  as deadline-scheduled "filler" inside the attention block loop, so the PE
  never idles at a phase boundary, never cold-throttles, and absorbs the
  Act-engine exp latency per block
- proj psum in the qkv [128,512] pool so scores double-buffering in ps_s is
  never blocked by projection; y/den psum single-buffered
- normalization pipelined in 3 stages (yu copy + den scatter / recip +
  gather + gpsimd broadcast / y muls) spread over following segments so no
  engine FIFO ever head-blocks on the chain's DMA or gpsimd latency
- input DMAs split across the SP and Activation hwdge queues in need-order;
  early-round norm DMAs ride the act queue while sync drains weights
- tail: last-segment reciprocal broadcast via a K=1 PE matmul, proj(j3)
  partials for hp0..2 emitted during the Act-bound final segment and the
  normalization chain, so only the hp3 contributions remain serial
"""


import numpy as np
import concourse.bass as bass
import concourse.tile as tile
from concourse import mybir, bacc

F32 = mybir.dt.float32
BF16 = mybir.dt.bfloat16


def build(T=2048, HL=8, C=1024):
    D = 64
    HP = HL // 2               # head pairs per core
    NCK = C // 128             # contraction chunks for qkv
    NI = T // 512              # 512-wide query blocks
    NTK = T // 128             # 128-wide key blocks

    nc = bacc.Bacc("TRN2", debug=False, num_devices=8)

    xt = nc.dram_tensor("xt", [NCK, 128, T], BF16, kind="ExternalInput")
    wq = nc.dram_tensor("wq", [NCK, 128, HL * D], BF16, kind="ExternalInput")
    wk = nc.dram_tensor("wk", [NCK, 128, HL * D], BF16, kind="ExternalInput")
    wv = nc.dram_tensor("wv", [NCK, 128, HL * D], BF16, kind="ExternalInput")
    wp = nc.dram_tensor("wp", [HP, 128, C], BF16, kind="ExternalInput")
    tri = nc.dram_tensor("tri", [128, 256], BF16, kind="ExternalInput")
    out = nc.dram_tensor("out", [T, C], F32, kind="ExternalOutput")

    with tile.TileContext(nc) as tc:
        with (
            tc.tile_pool(name="persist", bufs=1) as pers,
            tc.tile_pool(name="wqk", bufs=1) as wqkp,
            tc.tile_pool(name="att", bufs=12) as attp,
            tc.tile_pool(name="nrm", bufs=2) as nrm,
            tc.tile_pool(name="outp", bufs=4) as outp,
            tc.tile_pool(name="ps_mm", bufs=2, space="PSUM") as ps_mm,
            tc.tile_pool(name="ps_s", bufs=2, space="PSUM") as ps_s,
            tc.tile_pool(name="ps_y", bufs=1, space="PSUM") as ps_y,
        ):
            # ---- persistent SBUF ----
            xt_sb = pers.tile([128, NCK, T], BF16, tag="xt")
            q_sb = pers.tile([128, HP, T], BF16, tag="q")
            k_sb = pers.tile([128, HP, T], BF16, tag="k")
            v_sb = pers.tile([128, HP, NTK, 130], BF16, tag="v")
            wv_sb = pers.tile([128, NCK, HL * D], BF16, tag="wv")
            wp_sb = pers.tile([128, HP, C], BF16, tag="wp")
            tri_sb = pers.tile([128, 256], BF16, tag="tri")
            y_t = [pers.tile([128, T], BF16, tag=f"y{hp}", name=f"y{hp}")
                   for hp in range(HP)]
            wqk_t = {}
            for hp in range(HP):
                wqk_t[(0, hp)] = wqkp.tile([128, NCK, 128], BF16,
                                           tag=f"wq{hp}", name=f"wq{hp}")
                wqk_t[(1, hp)] = wqkp.tile([128, NCK, 128], BF16,
                                           tag=f"wk{hp}", name=f"wk{hp}")

            ones1 = pers.tile([1, 64], BF16, tag="ones1")
            nc.vector.memset(ones1[:], 1.0)
            nc.vector.memset(v_sb[:, :, :, 64:65], 1.0)
            nc.vector.memset(v_sb[:, :, :, 129:130], 1.0)

            # ---- DMA staging (order matters: queue drains in order) ----
            def dma_w(qk, hp):
                src = wq if qk == 0 else wk
                hs = slice(hp * 128, hp * 128 + 128)
                nc.sync.dma_start(wqk_t[(qk, hp)][:],
                                  src[:, :, hs].transpose([1, 0, 2]))

            def dma_x(win, eng=None):
                eng = eng or nc.sync
                wsl = slice(T // 4 * win, T // 4 * (win + 1))
                for ck in range(NCK):
                    eng.dma_start(xt_sb[:, ck, wsl], xt[ck, :, wsl])

            # sync queue: weights in need-order; act queue (idle at start):
            # the first two x windows. Later x windows + wp drain on sync
            # behind the weights, finishing long before they are needed.
            dma_w(0, 0)
            dma_w(1, 0)
            dma_x(0, nc.scalar)
            for ck in range(NCK):
                nc.sync.dma_start(wv_sb[:, ck, :], wv[ck])
            nc.sync.dma_start(tri_sb[:], tri[:])
            dma_x(1, nc.scalar)
            for hp in (1, 2, 3):
                dma_w(0, hp)
                dma_w(1, hp)
            dma_x(2)
            dma_x(3)
            for hp in range(HP):
                nc.sync.dma_start(wp_sb[:, hp, :], wp[hp])

            # ---- phase-A work units (emitted inline or as filler) ----
            # NOTE: each unit opens AND closes its psum tile atomically, so
            # arbitrary interleaving of units never splits an open
            # accumulation across other ps_mm.tile() rotations.
            def qk_tile(qk, hp, i):
                dst = q_sb if qk == 0 else k_sb
                ts = slice(512 * i, 512 * i + 512)
                w_h = wqk_t[(qk, hp)]
                p = ps_mm.tile([128, 512], F32, tag="mm", name=f"p{qk}{hp}{i}")
                for ck in range(NCK):
                    nc.tensor.matmul(p[:], w_h[:, ck, :], xt_sb[:, ck, ts],
                                     start=(ck == 0), stop=(ck == NCK - 1),
                                     skip_group_check=True)
                nc.vector.tensor_copy(dst[:, hp, ts], p[:])

            def v_tile(t):
                # token-tile t (128 keys), all head pairs at once
                ts = slice(128 * t, 128 * t + 128)
                p = ps_mm.tile([128, 512], F32, tag="mm", name=f"pv{t}")
                for ck in range(NCK):
                    nc.tensor.matmul(p[:], xt_sb[:, ck, ts], wv_sb[:, ck, :],
                                     start=(ck == 0), stop=(ck == NCK - 1),
                                     skip_group_check=True)
                pv = p[:].rearrange("p (h c) -> p h c", h=HP)
                nc.vector.tensor_copy(v_sb[:, :, t, 0:64], pv[:, :, 0:64])
                nc.vector.tensor_copy(v_sb[:, :, t, 65:129], pv[:, :, 64:128])

            # ---- prelude: enough phase A for (j0, hp0) ----
            for qk in (0, 1):
                qk_tile(qk, 0, 0)
            for t in range(2):
                v_tile(t)

            # ---- filler schedule: (j, hp) -> list of closures ----
            def QK(qk, hp, i):
                return [lambda: qk_tile(qk, hp, i)]

            def V(t):
                return [lambda: v_tile(t)]

            fill = {}
            fill[(0, 0)] = V(2) + V(3) + QK(0, 1, 0) + QK(1, 1, 0)
            fill[(0, 1)] = QK(0, 2, 0) + QK(1, 2, 0)
            fill[(0, 2)] = QK(0, 3, 0) + QK(1, 3, 0)
            fill[(0, 3)] = QK(0, 0, 1) + QK(1, 0, 1) + V(4) + V(5)
            fill[(1, 0)] = V(6) + V(7) + QK(0, 1, 1) + QK(1, 1, 1)
            fill[(1, 1)] = QK(0, 2, 1) + QK(1, 2, 1)
            fill[(1, 2)] = QK(0, 3, 1) + QK(1, 3, 1)
            fill[(1, 3)] = QK(0, 0, 2) + QK(1, 0, 2) + V(8) + V(9)
            fill[(2, 0)] = V(10) + V(11) + QK(0, 1, 2) + QK(1, 1, 2)
            fill[(2, 1)] = QK(0, 2, 2) + QK(1, 2, 2)
            fill[(2, 2)] = QK(0, 3, 2) + QK(1, 3, 2)
            fill[(2, 3)] = QK(0, 0, 3) + QK(1, 0, 3) + V(12) + V(13)
            fill[(3, 0)] = V(14) + V(15) + QK(0, 1, 3) + QK(1, 1, 3)
            fill[(3, 1)] = QK(0, 2, 3) + QK(1, 2, 3)
            fill[(3, 2)] = QK(0, 3, 3) + QK(1, 3, 3)
            fill[(3, 3)] = []

            def emit_proj(j, fs):
                # projection for query blocks fs of block j (y_sb ready)
                for f in fs:
                    t = 4 * j + f
                    ysl = slice(128 * t, 128 * t + 128)
                    ot = outp.tile([128, C], F32, tag="ot")
                    for ch in range(C // 512):
                        po = ps_mm.tile([128, 512], F32, tag="mm",
                                        name=f"po{t}_{ch}")
                        for hp2 in range(HP):
                            nc.tensor.matmul(po[:],
                                             y_t[hp2][:, ysl],
                                             wp_sb[:, hp2, 512 * ch:512 * ch + 512],
                                             start=(hp2 == 0), stop=(hp2 == HP - 1),
                                             skip_group_check=True)
                        nc.vector.tensor_copy(ot[:, 512 * ch:512 * ch + 512], po[:])
                    nc.sync.dma_start(out[128 * t:128 * t + 128, :], ot[:])

            # ---- attention + projection, with filler interleaved ----
            pending = None
            deferred2 = None
            stage3_q = []
            tail_ctx = None
            tail_po = []
            for j in range(NI):
                ntk = 4 * j + 4
                for hp in range(HP):
                    seg = list(fill[(j, hp)])
                    nseg = len(seg)
                    spread = max(1, (3 * ntk) // 4)

                    def pump(tkb, seg=seg, nseg=nseg, spread=spread):
                        want = (nseg * (tkb + 1) + spread - 1) // spread
                        while seg and (nseg - len(seg)) < min(want, nseg):
                            seg.pop(0)()

                    pyd = ps_y.tile([128, 1024], F32, tag="yd")
                    att_tiles = {}

                    def emit_attv(tkb, att_tiles=att_tiles, pyd=pyd,
                                  hp=hp, ntk=ntk, j=j):
                        r = tkb - 4 * j
                        co = 128 * r if r > 0 else 0
                        att = att_tiles.pop(tkb)
                        st = (tkb == 0)
                        sp = (tkb == ntk - 1)
                        nc.tensor.matmul(pyd[0:65, co:512],
                                         v_sb[:, hp, tkb, 0:65],
                                         att[:, 0, co:512], start=st, stop=sp,
                                         skip_group_check=True)
                        nc.tensor.matmul(pyd[0:65, 512 + co:1024],
                                         v_sb[:, hp, tkb, 65:130],
                                         att[:, 1, co:512], start=st, stop=sp,
                                         skip_group_check=True)

                    for tkb in range(ntk):
                        r = tkb - 4 * j
                        co = 128 * r if r > 0 else 0
                        ks = slice(128 * tkb, 128 * tkb + 128)
                        qs = slice(512 * j + co, 512 * j + 512)
                        pss = ps_s.tile([128, 1024], F32, tag="s")
                        nc.tensor.matmul(pss[:, co:512], k_sb[0:64, hp, ks],
                                         q_sb[0:64, hp, qs],
                                         start=True, stop=True, tile_position=(0, 0),
                                         skip_group_check=True)
                        nc.tensor.matmul(pss[:, 512 + co:1024], k_sb[64:128, hp, ks],
                                         q_sb[64:128, hp, qs],
                                         start=True, stop=True, tile_position=(64, 0),
                                         skip_group_check=True)
                        att = attp.tile([128, 2, 512], BF16, tag="att")
                        att_tiles[tkb] = att
                        pv2 = pss[:].rearrange("p (h t) -> p h t", h=2)
                        nc.scalar.activation(
                            att[:, :, co:512], pv2[:, :, co:512],
                            mybir.ActivationFunctionType.Exp, scale=0.125)
                        if r >= 0:
                            nc.vector.tensor_mul(
                                att[:, :, co:co + 128],
                                att[:, :, co:co + 128],
                                tri_sb[:].rearrange("p (h t) -> p h t", h=2))
                        # defer previous block's tail into this block's score
                        # stream so Act never idles; batch att*v per 2 key
                        # blocks to halve PE mode switches
                        if tkb == 0 and pending is not None:
                            pending()
                            pending = None
                        if tkb == 1 or tkb == 8:
                            for e in list(stage3_q):
                                if e[0] == 0:
                                    e[1]()
                                    stage3_q.remove(e)
                                elif tkb == 1:
                                    e[0] -= 1
                        if tkb == 2 and deferred2 is not None:
                            deferred2()
                            deferred2 = None
                        if tkb % 2 == 1:
                            if tkb >= 3:
                                emit_attv(tkb - 3)
                                emit_attv(tkb - 2)
                            if j > 0 and hp == 1 and tkb == 3:
                                emit_proj(j - 1, (0, 1))
                            if j > 0 and hp == 2 and tkb == 3:
                                emit_proj(j - 1, (2, 3))
                        if j == NI - 1 and hp == HP - 1 and tkb in (3, 6, 10):
                            # proj(j3, t0) partials as each hp's y lands;
                            # fills this Act-bound segment and shrinks the
                            # tail to the hp3 contributions only
                            hp2 = {3: 0, 6: 1, 10: 2}[tkb]
                            ysl = slice(128 * 4 * j, 128 * 4 * j + 128)
                            for ch in (0, 1):
                                if hp2 == 0:
                                    tail_po.append(ps_mm.tile(
                                        [128, 512], F32, tag="mm",
                                        name=f"pot0{ch}"))
                                nc.tensor.matmul(
                                    tail_po[ch][:], y_t[hp2][:, ysl],
                                    wp_sb[:, hp2, 512 * ch:512 * ch + 512],
                                    start=(hp2 == 0), stop=False,
                                    skip_group_check=True)
                        pump(tkb)

                    def emit_tail(emit_attv=emit_attv, pyd=pyd, hp=hp,
                                  ntk=ntk, j=j,
                                  last=(j == NI - 1 and hp == HP - 1)):
                        nonlocal deferred2, tail_ctx
                        emit_attv(ntk - 2)
                        emit_attv(ntk - 1)

                        # ---- normalization, stage 1 ----
                        # yu rows 0:64 = unnormalized y, row 64 = dens;
                        # dens scattered [1,1024]->[128,8] so the reciprocal
                        # runs 128-partition-parallel.  Early rounds route
                        # norm DMAs to the act hwdge queue: the sync queue is
                        # still draining input weights then, and act has
                        # surplus in the PE-bound rounds.
                        dmae = nc.scalar if j <= 1 else nc.sync
                        yu = nrm.tile([65, 1024], BF16, tag="yu")
                        nc.vector.tensor_copy(yu[:], pyd[0:65, :])
                        den8 = nrm.tile([128, 8], BF16, tag="den8")
                        dmae.dma_start(den8[:], yu[64:65, :])

                        if last:
                            def stage2t(den8=den8):
                                rec8 = nrm.tile([128, 8], BF16, tag="rec8b")
                                with nc.allow_low_precision(
                                        reason="bf16 recip feeds PE bcast"):
                                    nc.vector.reciprocal(rec8[:], den8[:])
                                recb = nrm.tile([1, 1024], BF16, tag="recb")
                                nc.sync.dma_start(recb[:], rec8[:])
                                return recb
                            tail_ctx = (yu, stage2t)
                            return

                        # stage 2 (deferred 2 blocks): recip + gather + bcast
                        def stage2(den8=den8, dmae=dmae):
                            rec8 = nrm.tile([128, 8], F32, tag="rec8")
                            nc.vector.reciprocal(rec8[:], den8[:])
                            rec = nrm.tile([1, 1024], F32, tag="rec")
                            dmae.dma_start(rec[:], rec8[:])
                            dT = nrm.tile([64, 1024], F32, tag="dT")
                            nc.gpsimd.partition_broadcast(dT[:], rec[0:1, :])
                            return dT

                        # stage 3 (deferred two segments): the y scaling,
                        # past the gpsimd latency so the muls never
                        # head-block the DVE queue
                        def stage3(yu=yu, hp=hp, j=j, dmae=dmae):
                            dT = stage3.dT
                            tqs = slice(512 * j, 512 * j + 512)
                            nc.vector.tensor_mul(y_t[hp][0:64, tqs],
                                                 yu[0:64, 0:512], dT[:, 0:512])
                            yu2 = nrm.tile([64, 512], BF16, tag="yu2")
                            nc.vector.tensor_mul(yu2[:], yu[0:64, 512:1024],
                                                 dT[:, 512:1024])
                            dmae.dma_start(y_t[hp][64:128, tqs], yu2[:])

                        def run_stage2(stage2=stage2, stage3=stage3):
                            stage3.dT = stage2()

                        deferred2 = run_stage2
                        # deeper deferral for the short j0 segments: their
                        # norm chains are still DMA-latency-bound at startup
                        stage3_q.append([2 if (j == 0 and hp < 3) else 1,
                                         stage3])

                    pending = emit_tail

            # ---- tail: last normalization via PE broadcast, with proj(j3)
            # partials for hp0..2 overlapping the whole chain so the PE
            # stays warm and only the hp3 contributions remain at the end
            for e in stage3_q:
                e[1]()
            stage3_q = []
            pending()
            yu, stage2t = tail_ctx

            def proj_part(po, t, ch, hps, start, stop):
                ysl = slice(128 * t, 128 * t + 128)
                for hp2 in hps:
                    nc.tensor.matmul(po[:, 512 * ch:512 * ch + 512],
                                     y_t[hp2][:, ysl],
                                     wp_sb[:, hp2, 512 * ch:512 * ch + 512],
                                     start=(start and hp2 == hps[0]),
                                     stop=(stop and hp2 == hps[-1]),
                                     skip_group_check=True)

            tb = 4 * (NI - 1)
            po_t = {}
            for ch in (0, 1):
                po_t[(0, ch)] = tail_po[ch]
            po_t[1] = ps_s.tile([128, 1024], F32, tag="s", name="pot1")
            po_t[2] = ps_s.tile([128, 1024], F32, tag="s", name="pot2")
            # t1/t2 hp0..2 partials (PE work during the normalization chain)
            for t in (1, 2):
                proj_part(po_t[t], tb + t, 0, (0, 1, 2), True, False)
                proj_part(po_t[t], tb + t, 1, (0, 1, 2), True, False)
            recb = stage2t()
            pbc = ps_y.tile([128, 1024], F32, tag="yd", name="pbc")
            nc.tensor.matmul(pbc[0:64, 0:512], ones1[0:1, :], recb[0:1, 0:512],
                             start=True, stop=True, skip_group_check=True)
            nc.tensor.matmul(pbc[0:64, 512:1024], ones1[0:1, :],
                             recb[0:1, 512:1024],
                             start=True, stop=True, skip_group_check=True)
            tqs = slice(512 * (NI - 1), 512 * NI)
            nc.vector.tensor_mul(y_t[HP - 1][0:64, tqs],
                                 yu[0:64, 0:512], pbc[0:64, 0:512])
            yu2 = nrm.tile([64, 512], BF16, tag="yu2")
            nc.vector.tensor_mul(yu2[:], yu[0:64, 512:1024],
                                 pbc[0:64, 512:1024])
            nc.sync.dma_start(y_t[HP - 1][64:128, tqs], yu2[:])

            # hp3 contributions + copy-out
            ot0 = outp.tile([128, C], F32, tag="ot")
            ysl = slice(128 * tb, 128 * tb + 128)
            for ch in (0, 1):
                nc.tensor.matmul(po_t[(0, ch)][:], y_t[HP - 1][:, ysl],
                                 wp_sb[:, HP - 1, 512 * ch:512 * ch + 512],
                                 start=False, stop=True, skip_group_check=True)
                nc.vector.tensor_copy(ot0[:, 512 * ch:512 * ch + 512],
                                      po_t[(0, ch)][:])
            nc.sync.dma_start(out[128 * tb:128 * tb + 128, :], ot0[:])
            for t in (1, 2):
                proj_part(po_t[t], tb + t, 0, (3,), False, True)
                proj_part(po_t[t], tb + t, 1, (3,), False, True)
                ot = outp.tile([128, C], F32, tag="ot")
                nc.vector.tensor_copy(ot[:, 0:512], po_t[t][:, 0:512])
                nc.vector.tensor_copy(ot[:, 512:1024], po_t[t][:, 512:1024])
                nc.sync.dma_start(out[128 * (tb + t):128 * (tb + t) + 128, :],
                                  ot[:])
            emit_proj(NI - 1, (3,))

    nc.compile()
    return nc


def make_inputs(x_b, w_qkv, w_proj, g, HL=8):
    """Host-side prep of one core's input map.

    x_b: [T, C] fp32 (one batch), g: head-group index (0 or 1).
    """
    import ml_dtypes
    BF = ml_dtypes.bfloat16
    T, C = x_b.shape
    D = 64
    NCK = C // 128
    HP = HL // 2
    h0 = g * HL * D
    xt = np.ascontiguousarray(x_b.T.reshape(NCK, 128, T)).astype(BF)
    wq = np.ascontiguousarray(
        w_qkv[:, h0:h0 + HL * D].reshape(NCK, 128, HL * D)).astype(BF)
    wk = np.ascontiguousarray(
        w_qkv[:, C + h0:C + h0 + HL * D].reshape(NCK, 128, HL * D)).astype(BF)
    wv = np.ascontiguousarray(
        w_qkv[:, 2 * C + h0:2 * C + h0 + HL * D].reshape(NCK, 128, HL * D)).astype(BF)
    wp = np.ascontiguousarray(
        w_proj[h0:h0 + HL * D, :].reshape(HP, 128, C)).astype(BF)
    t1 = np.triu(np.ones((128, 128), dtype=np.float32))
    tri = np.concatenate([t1, t1], axis=1).astype(BF)
    return {"xt": xt, "wq": wq, "wk": wk, "wv": wv, "wp": wp, "tri": tri}


_NC_CACHE = {}


def kernel(x, w_qkv, w_proj):
    import numpy as np
    from concourse.bass_utils import run_bass_kernel_spmd

    x = np.ascontiguousarray(np.asarray(x, dtype=np.float32))
    w_qkv = np.ascontiguousarray(np.asarray(w_qkv, dtype=np.float32))
    w_proj = np.ascontiguousarray(np.asarray(w_proj, dtype=np.float32))
    B, T, C = x.shape

    key = (T, C)
    if key not in _NC_CACHE:
        _NC_CACHE[key] = build(T=T, HL=8, C=C)
    nc = _NC_CACHE[key]

    in_maps = [make_inputs(x[c // 2], w_qkv, w_proj, c % 2, HL=8) for c in range(8)]
    res = run_bass_kernel_spmd(nc, in_maps, core_ids=list(range(8)), trace=False)

    out = np.zeros((B, T, C), dtype=np.float32)
    for c in range(8):
        out[c // 2] += res.results[c]["out"]
    return out


# revision 34
# speedup vs baseline: 1.0091x; 1.0091x over previous
"""Causal self-attention kernel v8 for 8 Trainium2 NeuronCores (Bass/Tile).

B=4, T=2048, C=1024, 16 heads. 8 cores = 4 batches x 2 head-groups (8 heads
each); host sums the two projection partials per batch.

Design (vs the 330913ns v3 baseline):
- v computed pre-transposed (lhsT = x^T chunks, rhs = w_v) -> kills the 64
  PE transposes + extra copies; v psum lands directly in [keys, d] layout
- single fused schedule: qkv tile matmuls for later head-pairs are emitted
  as deadline-scheduled "filler" inside the attention block loop, so the PE
  never idles at a phase boundary, never cold-throttles, and absorbs the
  Act-engine exp latency per block
- proj psum in the qkv [128,512] pool so scores double-buffering in ps_s is
  never blocked by projection; y/den psum single-buffered
- normalization pipelined in 3 stages (yu copy + den scatter / recip +
  gather + gpsimd broadcast / y muls) spread over following segments so no
  engine FIFO ever head-blocks on the chain's DMA or gpsimd latency
- input DMAs split across the SP and Activation hwdge queues in need-order;
  early-round norm DMAs ride the act queue while sync drains weights
- tail: last-segment reciprocal broadcast via a K=1 PE matmul, proj(j3)
  partials for hp0..2 emitted during the Act-bound final segment and the
  normalization chain, so only the hp3 contributions remain serial
"""


import numpy as np
import concourse.bass as bass
import concourse.tile as tile
from concourse import mybir, bacc

F32 = mybir.dt.float32
BF16 = mybir.dt.bfloat16


def build(T=2048, HL=8, C=1024):
    D = 64
    HP = HL // 2               # head pairs per core
    NCK = C // 128             # contraction chunks for qkv
    NI = T // 512              # 512-wide query blocks
    NTK = T // 128             # 128-wide key blocks

    nc = bacc.Bacc("TRN2", debug=False, num_devices=8)

    xt = nc.dram_tensor("xt", [NCK, 128, T], BF16, kind="ExternalInput")
    wq = nc.dram_tensor("wq", [NCK, 128, HL * D], BF16, kind="ExternalInput")
    wk = nc.dram_tensor("wk", [NCK, 128, HL * D], BF16, kind="ExternalInput")
    wv = nc.dram_tensor("wv", [NCK, 128, HL * D], BF16, kind="ExternalInput")
    wp = nc.dram_tensor("wp", [HP, 128, C], BF16, kind="ExternalInput")
    tri = nc.dram_tensor("tri", [128, 256], BF16, kind="ExternalInput")
    out = nc.dram_tensor("out", [T, C], F32, kind="ExternalOutput")

    with tile.TileContext(nc) as tc:
        with (
            tc.tile_pool(name="persist", bufs=1) as pers,
            tc.tile_pool(name="wqk", bufs=1) as wqkp,
            tc.tile_pool(name="att", bufs=12) as attp,
            tc.tile_pool(name="nrm", bufs=2) as nrm,
            tc.tile_pool(name="outp", bufs=4) as outp,
            tc.tile_pool(name="ps_mm", bufs=2, space="PSUM") as ps_mm,
            tc.tile_pool(name="ps_s", bufs=2, space="PSUM") as ps_s,
            tc.tile_pool(name="ps_y", bufs=1, space="PSUM") as ps_y,
        ):
            # ---- persistent SBUF ----
            xt_sb = pers.tile([128, NCK, T], BF16, tag="xt")
            q_sb = pers.tile([128, HP, T], BF16, tag="q")
            k_sb = pers.tile([128, HP, T], BF16, tag="k")
            v_sb = pers.tile([128, HP, NTK, 130], BF16, tag="v")
            wv_sb = pers.tile([128, NCK, HL * D], BF16, tag="wv")
            wp_sb = pers.tile([128, HP, C], BF16, tag="wp")
            tri_sb = pers.tile([128, 256], BF16, tag="tri")
            y_t = [pers.tile([128, T], BF16, tag=f"y{hp}", name=f"y{hp}")
                   for hp in range(HP)]
            wqk_t = {}
            for hp in range(HP):
                wqk_t[(0, hp)] = wqkp.tile([128, NCK, 128], BF16,
                                           tag=f"wq{hp}", name=f"wq{hp}")
                wqk_t[(1, hp)] = wqkp.tile([128, NCK, 128], BF16,
                                           tag=f"wk{hp}", name=f"wk{hp}")

            ones1 = pers.tile([1, 64], BF16, tag="ones1")
            nc.vector.memset(ones1[:], 1.0)
            nc.vector.memset(v_sb[:, :, :, 64:65], 1.0)
            nc.vector.memset(v_sb[:, :, :, 129:130], 1.0)

            # ---- DMA staging (order matters: queue drains in order) ----
            def dma_w(qk, hp):
                src = wq if qk == 0 else wk
                hs = slice(hp * 128, hp * 128 + 128)
                nc.sync.dma_start(wqk_t[(qk, hp)][:],
                                  src[:, :, hs].transpose([1, 0, 2]))

            def dma_x(win, eng=None):
                eng = eng or nc.sync
                wsl = slice(T // 4 * win, T // 4 * (win + 1))
                for ck in range(NCK):
                    eng.dma_start(xt_sb[:, ck, wsl], xt[ck, :, wsl])

            # sync queue: weights in need-order; act queue (idle at start):
            # the first two x windows. Later x windows + wp drain on sync
            # behind the weights, finishing long before they are needed.
            dma_w(0, 0)
            dma_w(1, 0)
            dma_x(0, nc.scalar)
            for ck in range(NCK):
                nc.sync.dma_start(wv_sb[:, ck, :], wv[ck])
            nc.sync.dma_start(tri_sb[:], tri[:])
            dma_x(1, nc.scalar)
            for hp in (1, 2, 3):
                dma_w(0, hp)
                dma_w(1, hp)
            dma_x(2)
            dma_x(3)
            for hp in range(HP):
                nc.sync.dma_start(wp_sb[:, hp, :], wp[hp])

            # ---- phase-A work units (emitted inline or as filler) ----
            # NOTE: each unit opens AND closes its psum tile atomically, so
            # arbitrary interleaving of units never splits an open
            # accumulation across other ps_mm.tile() rotations.
            def qk_tile(qk, hp, i):
                dst = q_sb if qk == 0 else k_sb
                ts = slice(512 * i, 512 * i + 512)
                w_h = wqk_t[(qk, hp)]
                p = ps_mm.tile([128, 512], F32, tag="mm", name=f"p{qk}{hp}{i}")
                for ck in range(NCK):
                    nc.tensor.matmul(p[:], w_h[:, ck, :], xt_sb[:, ck, ts],
                                     start=(ck == 0), stop=(ck == NCK - 1),
                                     skip_group_check=True)
                nc.vector.tensor_copy(dst[:, hp, ts], p[:])

            def v_tile(t):
                # token-tile t (128 keys), all head pairs at once
                ts = slice(128 * t, 128 * t + 128)
                p = ps_mm.tile([128, 512], F32, tag="mm", name=f"pv{t}")
                for ck in range(NCK):
                    nc.tensor.matmul(p[:], xt_sb[:, ck, ts], wv_sb[:, ck, :],
                                     start=(ck == 0), stop=(ck == NCK - 1),
                                     skip_group_check=True)
                pv = p[:].rearrange("p (h c) -> p h c", h=HP)
                nc.vector.tensor_copy(v_sb[:, :, t, 0:64], pv[:, :, 0:64])
                nc.vector.tensor_copy(v_sb[:, :, t, 65:129], pv[:, :, 64:128])

            # ---- prelude: enough phase A for (j0, hp0) ----
            for qk in (0, 1):
                qk_tile(qk, 0, 0)
            for t in range(2):
                v_tile(t)

            # ---- filler schedule: (j, hp) -> list of closures ----
            def QK(qk, hp, i):
                return [lambda: qk_tile(qk, hp, i)]

            def V(t):
                return [lambda: v_tile(t)]

            # segment order interleaves j2/j3 so the Act-bound j3 segments
            # can absorb phase-A filler and proj(j2); proj_slots places each
            # projection where its y inputs are 2+ segments old
            seg_order = [(0, 0), (0, 1), (0, 2), (0, 3),
                         (1, 0), (1, 1), (1, 2), (1, 3),
                         (2, 0), (2, 1), (3, 0), (2, 2),
                         (3, 1), (2, 3), (3, 2), (3, 3)]
            proj_slots = {
                (1, 1): {3: (0, (0, 1))},
                (1, 2): {3: (0, (2, 3))},
                (2, 1): {3: (1, (0, 1))},
                (3, 1): {3: (1, (2, 3))},
                (3, 3): {3: (2, (0, 1)), 5: (2, (2, 3))},
            }
            fill = {}
            fill[(0, 0)] = V(2) + V(3) + QK(0, 1, 0) + QK(1, 1, 0)
            fill[(0, 1)] = QK(0, 2, 0) + QK(1, 2, 0)
            fill[(0, 2)] = QK(0, 3, 0) + QK(1, 3, 0)
            fill[(0, 3)] = QK(0, 0, 1) + QK(1, 0, 1) + V(4) + V(5)
            fill[(1, 0)] = V(6) + V(7) + QK(0, 1, 1) + QK(1, 1, 1)
            fill[(1, 1)] = QK(0, 2, 1) + QK(1, 2, 1)
            fill[(1, 2)] = QK(0, 3, 1) + QK(1, 3, 1)
            fill[(1, 3)] = QK(0, 0, 2) + QK(1, 0, 2) + V(8) + V(9)
            fill[(2, 0)] = V(10) + V(11) + QK(0, 1, 2) + QK(1, 1, 2)
            fill[(2, 1)] = QK(0, 0, 3) + QK(1, 0, 3)
            fill[(3, 0)] = V(12) + V(13) + V(14) + V(15) \
                + QK(0, 2, 2) + QK(1, 2, 2)
            fill[(2, 2)] = QK(0, 1, 3) + QK(1, 1, 3)
            fill[(3, 1)] = QK(0, 3, 2) + QK(1, 3, 2)
            fill[(2, 3)] = QK(0, 2, 3) + QK(1, 2, 3)
            fill[(3, 2)] = QK(0, 3, 3) + QK(1, 3, 3)
            fill[(3, 3)] = []

            def emit_proj(j, fs):
                # projection for query blocks fs of block j (y_sb ready)
                for f in fs:
                    t = 4 * j + f
                    ysl = slice(128 * t, 128 * t + 128)
                    ot = outp.tile([128, C], F32, tag="ot")
                    for ch in range(C // 512):
                        po = ps_mm.tile([128, 512], F32, tag="mm",
                                        name=f"po{t}_{ch}")
                        for hp2 in range(HP):
                            nc.tensor.matmul(po[:],
                                             y_t[hp2][:, ysl],
                                             wp_sb[:, hp2, 512 * ch:512 * ch + 512],
                                             start=(hp2 == 0), stop=(hp2 == HP - 1),
                                             skip_group_check=True)
                        nc.vector.tensor_copy(ot[:, 512 * ch:512 * ch + 512], po[:])
                    nc.sync.dma_start(out[128 * t:128 * t + 128, :], ot[:])

            # ---- attention + projection, with filler interleaved ----
            pending = None
            deferred2 = None
            stage3_q = []
            tail_ctx = None
            for j, hp in seg_order:
                ntk = 4 * j + 4
                if True:
                    seg = list(fill[(j, hp)])
                    nseg = len(seg)
                    spread = max(1, (3 * ntk) // 4)

                    def pump(tkb, seg=seg, nseg=nseg, spread=spread):
                        want = (nseg * (tkb + 1) + spread - 1) // spread
                        while seg and (nseg - len(seg)) < min(want, nseg):
                            seg.pop(0)()

                    pyd = ps_y.tile([128, 1024], F32, tag="yd")
                    att_tiles = {}

                    def emit_attv(tkb, att_tiles=att_tiles, pyd=pyd,
                                  hp=hp, ntk=ntk, j=j):
                        r = tkb - 4 * j
                        co = 128 * r if r > 0 else 0
                        att = att_tiles.pop(tkb)
                        st = (tkb == 0)
                        sp = (tkb == ntk - 1)
                        nc.tensor.matmul(pyd[0:65, co:512],
                                         v_sb[:, hp, tkb, 0:65],
                                         att[:, 0, co:512], start=st, stop=sp,
                                         skip_group_check=True)
                        nc.tensor.matmul(pyd[0:65, 512 + co:1024],
                                         v_sb[:, hp, tkb, 65:130],
                                         att[:, 1, co:512], start=st, stop=sp,
                                         skip_group_check=True)

                    for tkb in range(ntk):
                        r = tkb - 4 * j
                        co = 128 * r if r > 0 else 0
                        ks = slice(128 * tkb, 128 * tkb + 128)
                        qs = slice(512 * j + co, 512 * j + 512)
                        pss = ps_s.tile([128, 1024], F32, tag="s")
                        nc.tensor.matmul(pss[:, co:512], k_sb[0:64, hp, ks],
                                         q_sb[0:64, hp, qs],
                                         start=True, stop=True, tile_position=(0, 0),
                                         skip_group_check=True)
                        nc.tensor.matmul(pss[:, 512 + co:1024], k_sb[64:128, hp, ks],
                                         q_sb[64:128, hp, qs],
                                         start=True, stop=True, tile_position=(64, 0),
                                         skip_group_check=True)
                        att = attp.tile([128, 2, 512], BF16, tag="att")
                        att_tiles[tkb] = att
                        pv2 = pss[:].rearrange("p (h t) -> p h t", h=2)
                        nc.scalar.activation(
                            att[:, :, co:512], pv2[:, :, co:512],
                            mybir.ActivationFunctionType.Exp, scale=0.125)
                        if r >= 0:
                            nc.vector.tensor_mul(
                                att[:, :, co:co + 128],
                                att[:, :, co:co + 128],
                                tri_sb[:].rearrange("p (h t) -> p h t", h=2))
                        # defer previous block's tail into this block's score
                        # stream so Act never idles; batch att*v per 2 key
                        # blocks to halve PE mode switches
                        if tkb == 0 and pending is not None:
                            pending()
                            pending = None
                        if tkb == 1 or tkb == 8:
                            for e in list(stage3_q):
                                if e[0] == 0:
                                    e[1]()
                                    stage3_q.remove(e)
                                elif tkb == 1:
                                    e[0] -= 1
                        if tkb == 2 and deferred2 is not None:
                            deferred2()
                            deferred2 = None
                        if tkb % 2 == 1:
                            if tkb >= 3:
                                emit_attv(tkb - 3)
                                emit_attv(tkb - 2)
                            ps = proj_slots.get((j, hp), {}).get(tkb)
                            if ps is not None:
                                emit_proj(ps[0], ps[1])
                        pump(tkb)

                    def emit_tail(emit_attv=emit_attv, pyd=pyd, hp=hp,
                                  ntk=ntk, j=j,
                                  last=(j == NI - 1 and hp == HP - 1)):
                        nonlocal deferred2, tail_ctx
                        emit_attv(ntk - 2)
                        emit_attv(ntk - 1)

                        # ---- normalization, stage 1 ----
                        # yu rows 0:64 = unnormalized y, row 64 = dens;
                        # dens scattered [1,1024]->[128,8] so the reciprocal
                        # runs 128-partition-parallel.  Early rounds route
                        # norm DMAs to the act hwdge queue: the sync queue is
                        # still draining input weights then, and act has
                        # surplus in the PE-bound rounds.
                        dmae = nc.scalar if j <= 1 else nc.sync
                        yu = nrm.tile([65, 1024], BF16, tag="yu")
                        nc.vector.tensor_copy(yu[:], pyd[0:65, :])
                        den8 = nrm.tile([128, 8], BF16, tag="den8")
                        dmae.dma_start(den8[:], yu[64:65, :])

                        if last:
                            def stage2t(den8=den8):
                                rec8 = nrm.tile([128, 8], BF16, tag="rec8b")
                                with nc.allow_low_precision(
                                        reason="bf16 recip feeds PE bcast"):
                                    nc.vector.reciprocal(rec8[:], den8[:])
                                recb = nrm.tile([1, 1024], BF16, tag="recb")
                                nc.sync.dma_start(recb[:], rec8[:])
                                return recb
                            tail_ctx = (yu, stage2t)
                            return

                        # stage 2 (deferred 2 blocks): recip + gather + bcast
                        def stage2(den8=den8, dmae=dmae):
                            rec8 = nrm.tile([128, 8], F32, tag="rec8")
                            nc.vector.reciprocal(rec8[:], den8[:])
                            rec = nrm.tile([1, 1024], F32, tag="rec")
                            dmae.dma_start(rec[:], rec8[:])
                            dT = nrm.tile([64, 1024], F32, tag="dT")
                            nc.gpsimd.partition_broadcast(dT[:], rec[0:1, :])
                            return dT

                        # stage 3 (deferred two segments): the y scaling,
                        # past the gpsimd latency so the muls never
                        # head-block the DVE queue
                        def stage3(yu=yu, hp=hp, j=j, dmae=dmae):
                            dT = stage3.dT
                            tqs = slice(512 * j, 512 * j + 512)
                            nc.vector.tensor_mul(y_t[hp][0:64, tqs],
                                                 yu[0:64, 0:512], dT[:, 0:512])
                            yu2 = nrm.tile([64, 512], BF16, tag="yu2")
                            nc.vector.tensor_mul(yu2[:], yu[0:64, 512:1024],
                                                 dT[:, 512:1024])
                            dmae.dma_start(y_t[hp][64:128, tqs], yu2[:])

                        def run_stage2(stage2=stage2, stage3=stage3):
                            stage3.dT = stage2()

                        deferred2 = run_stage2
                        # deeper deferral for the short j0 segments: their
                        # norm chains are still DMA-latency-bound at startup
                        stage3_q.append([2 if (j == 0 and hp < 3) else 1,
                                         stage3])

                    pending = emit_tail

            # ---- tail: last normalization via PE broadcast, with proj(j3)
            # partials for hp0..2 overlapping the whole chain so the PE
            # stays warm and only the hp3 contributions remain at the end
            for e in stage3_q:
                e[1]()
            stage3_q = []
            pending()
            yu, stage2t = tail_ctx

            def proj_part(po, t, ch, hps, start, stop):
                ysl = slice(128 * t, 128 * t + 128)
                for hp2 in hps:
                    nc.tensor.matmul(po[:, 512 * ch:512 * ch + 512],
                                     y_t[hp2][:, ysl],
                                     wp_sb[:, hp2, 512 * ch:512 * ch + 512],
                                     start=(start and hp2 == hps[0]),
                                     stop=(stop and hp2 == hps[-1]),
                                     skip_group_check=True)

            tb = 4 * (NI - 1)
            po_t = {}
            ysl = slice(128 * tb, 128 * tb + 128)
            for ch in (0, 1):
                po = ps_mm.tile([128, 512], F32, tag="mm", name=f"pot0{ch}")
                for hp2 in (0, 1, 2):
                    nc.tensor.matmul(po[:], y_t[hp2][:, ysl],
                                     wp_sb[:, hp2, 512 * ch:512 * ch + 512],
                                     start=(hp2 == 0), stop=False,
                                     skip_group_check=True)
                po_t[(0, ch)] = po
            po_t[1] = ps_s.tile([128, 1024], F32, tag="s", name="pot1")
            po_t[2] = ps_s.tile([128, 1024], F32, tag="s", name="pot2")
            # t1/t2 hp0..2 partials (PE work during the normalization chain)
            for t in (1, 2):
                proj_part(po_t[t], tb + t, 0, (0, 1, 2), True, False)
                proj_part(po_t[t], tb + t, 1, (0, 1, 2), True, False)
            recb = stage2t()
            pbc = ps_y.tile([128, 1024], F32, tag="yd", name="pbc")
            nc.tensor.matmul(pbc[0:64, 0:512], ones1[0:1, :], recb[0:1, 0:512],
                             start=True, stop=True, skip_group_check=True)
            nc.tensor.matmul(pbc[0:64, 512:1024], ones1[0:1, :],
                             recb[0:1, 512:1024],
                             start=True, stop=True, skip_group_check=True)
            tqs = slice(512 * (NI - 1), 512 * NI)
            nc.vector.tensor_mul(y_t[HP - 1][0:64, tqs],
                                 yu[0:64, 0:512], pbc[0:64, 0:512])
            yu2 = nrm.tile([64, 512], BF16, tag="yu2")
            nc.vector.tensor_mul(yu2[:], yu[0:64, 512:1024],
                                 pbc[0:64, 512:1024])
            nc.sync.dma_start(y_t[HP - 1][64:128, tqs], yu2[:])

            # hp3 contributions + copy-out
            ot0 = outp.tile([128, C], F32, tag="ot")
            ysl = slice(128 * tb, 128 * tb + 128)
            for ch in (0, 1):
                nc.tensor.matmul(po_t[(0, ch)][:], y_t[HP - 1][:, ysl],
                                 wp_sb[:, HP - 1, 512 * ch:512 * ch + 512],
                                 start=False, stop=True, skip_group_check=True)
                nc.vector.tensor_copy(ot0[:, 512 * ch:512 * ch + 512],
                                      po_t[(0, ch)][:])
            nc.sync.dma_start(out[128 * tb:128 * tb + 128, :], ot0[:])
            for t in (1, 2):
                proj_part(po_t[t], tb + t, 0, (3,), False, True)
                proj_part(po_t[t], tb + t, 1, (3,), False, True)
                ot = outp.tile([128, C], F32, tag="ot")
                nc.vector.tensor_copy(ot[:, 0:512], po_t[t][:, 0:512])
                nc.vector.tensor_copy(ot[:, 512:1024], po_t[t][:, 512:1024])
                nc.sync.dma_start(out[128 * (tb + t):128 * (tb + t) + 128, :],
                                  ot[:])
            emit_proj(NI - 1, (3,))

    nc.compile()
    return nc


def make_inputs(x_b, w_qkv, w_proj, g, HL=8):
    """Host-side prep of one core's input map.

    x_b: [T, C] fp32 (one batch), g: head-group index (0 or 1).
    """
    import ml_dtypes
    BF = ml_dtypes.bfloat16
    T, C = x_b.shape
    D = 64
    NCK = C // 128
    HP = HL // 2
    h0 = g * HL * D
    xt = np.ascontiguousarray(x_b.T.reshape(NCK, 128, T)).astype(BF)
    wq = np.ascontiguousarray(
        w_qkv[:, h0:h0 + HL * D].reshape(NCK, 128, HL * D)).astype(BF)
    wk = np.ascontiguousarray(
        w_qkv[:, C + h0:C + h0 + HL * D].reshape(NCK, 128, HL * D)).astype(BF)
    wv = np.ascontiguousarray(
        w_qkv[:, 2 * C + h0:2 * C + h0 + HL * D].reshape(NCK, 128, HL * D)).astype(BF)
    wp = np.ascontiguousarray(
        w_proj[h0:h0 + HL * D, :].reshape(HP, 128, C)).astype(BF)
    t1 = np.triu(np.ones((128, 128), dtype=np.float32))
    tri = np.concatenate([t1, t1], axis=1).astype(BF)
    return {"xt": xt, "wq": wq, "wk": wk, "wv": wv, "wp": wp, "tri": tri}


_NC_CACHE = {}


def kernel(x, w_qkv, w_proj):
    import numpy as np
    from concourse.bass_utils import run_bass_kernel_spmd

    x = np.ascontiguousarray(np.asarray(x, dtype=np.float32))
    w_qkv = np.ascontiguousarray(np.asarray(w_qkv, dtype=np.float32))
    w_proj = np.ascontiguousarray(np.asarray(w_proj, dtype=np.float32))
    B, T, C = x.shape

    key = (T, C)
    if key not in _NC_CACHE:
        _NC_CACHE[key] = build(T=T, HL=8, C=C)
    nc = _NC_CACHE[key]

    in_maps = [make_inputs(x[c // 2], w_qkv, w_proj, c % 2, HL=8) for c in range(8)]
    res = run_bass_kernel_spmd(nc, in_maps, core_ids=list(range(8)), trace=False)

    out = np.zeros((B, T, C), dtype=np.float32)
    for c in range(8):
        out[c // 2] += res.results[c]["out"]
    return out


# revision 42
# speedup vs baseline: 1.0116x; 1.0024x over previous
"""Causal self-attention kernel v8 for 8 Trainium2 NeuronCores (Bass/Tile).

B=4, T=2048, C=1024, 16 heads. 8 cores = 4 batches x 2 head-groups (8 heads
each); host sums the two projection partials per batch.

Design (vs the 330913ns v3 baseline):
- v computed pre-transposed (lhsT = x^T chunks, rhs = w_v) -> kills the 64
  PE transposes + extra copies; v psum lands directly in [keys, d] layout
- single fused schedule: qkv tile matmuls for later head-pairs are emitted
  as deadline-scheduled "filler" inside the attention block loop, so the PE
  never idles at a phase boundary, never cold-throttles, and absorbs the
  Act-engine exp latency per block
- proj psum in the qkv [128,512] pool so scores double-buffering in ps_s is
  never blocked by projection; y/den psum single-buffered
- normalization pipelined in 3 stages (yu copy + den scatter / recip +
  gather + gpsimd broadcast / y muls) spread over following segments so no
  engine FIFO ever head-blocks on the chain's DMA or gpsimd latency
- input DMAs split across the SP and Activation hwdge queues in need-order;
  early-round norm DMAs ride the act queue while sync drains weights
- tail: last-segment reciprocal broadcast via a K=1 PE matmul, proj(j3)
  partials for hp0..2 emitted during the Act-bound final segment and the
  normalization chain, so only the hp3 contributions remain serial
"""


import numpy as np
import concourse.bass as bass
import concourse.tile as tile
from concourse import mybir, bacc

F32 = mybir.dt.float32
BF16 = mybir.dt.bfloat16


def build(T=2048, HL=8, C=1024):
    D = 64
    HP = HL // 2               # head pairs per core
    NCK = C // 128             # contraction chunks for qkv
    NI = T // 512              # 512-wide query blocks
    NTK = T // 128             # 128-wide key blocks

    nc = bacc.Bacc("TRN2", debug=False, num_devices=8)

    xt = nc.dram_tensor("xt", [NCK, 128, T], BF16, kind="ExternalInput")
    wq = nc.dram_tensor("wq", [NCK, 128, HL * D], BF16, kind="ExternalInput")
    wk = nc.dram_tensor("wk", [NCK, 128, HL * D], BF16, kind="ExternalInput")
    wv = nc.dram_tensor("wv", [NCK, 128, HL * D], BF16, kind="ExternalInput")
    wp = nc.dram_tensor("wp", [HP, 128, C], BF16, kind="ExternalInput")
    tri = nc.dram_tensor("tri", [128, 256], BF16, kind="ExternalInput")
    out = nc.dram_tensor("out", [T, C], F32, kind="ExternalOutput")

    with tile.TileContext(nc) as tc:
        with (
            tc.tile_pool(name="persist", bufs=1) as pers,
            tc.tile_pool(name="wqk", bufs=1) as wqkp,
            tc.tile_pool(name="att", bufs=12) as attp,
            tc.tile_pool(name="nrm", bufs=2) as nrm,
            tc.tile_pool(name="outp", bufs=3) as outp,
            tc.tile_pool(name="ps_mm", bufs=2, space="PSUM") as ps_mm,
            tc.tile_pool(name="ps_s", bufs=2, space="PSUM") as ps_s,
            tc.tile_pool(name="ps_y", bufs=1, space="PSUM") as ps_y,
        ):
            # ---- persistent SBUF ----
            xt_sb = pers.tile([128, NCK, T], BF16, tag="xt")
            q_sb = pers.tile([128, HP, T], BF16, tag="q")
            k_sb = pers.tile([128, HP, T], BF16, tag="k")
            v_sb = pers.tile([128, HP, NTK, 130], BF16, tag="v")
            wv_sb = pers.tile([128, NCK, HL * D], BF16, tag="wv")
            wp_sb = pers.tile([128, HP, C], BF16, tag="wp")
            tri_sb = pers.tile([128, 256], BF16, tag="tri")
            y_t = [pers.tile([128, T], BF16, tag=f"y{hp}", name=f"y{hp}")
                   for hp in range(HP)]
            wqk_t = {}
            for hp in range(HP):
                wqk_t[(0, hp)] = wqkp.tile([128, NCK, 128], BF16,
                                           tag=f"wq{hp}", name=f"wq{hp}")
                wqk_t[(1, hp)] = wqkp.tile([128, NCK, 128], BF16,
                                           tag=f"wk{hp}", name=f"wk{hp}")

            ones1 = pers.tile([1, 64], BF16, tag="ones1")
            nc.vector.memset(ones1[:], 1.0)
            nc.vector.memset(v_sb[:, :, :, 64:65], 1.0)
            nc.vector.memset(v_sb[:, :, :, 129:130], 1.0)

            # ---- DMA staging (order matters: queue drains in order) ----
            def dma_w(qk, hp):
                src = wq if qk == 0 else wk
                hs = slice(hp * 128, hp * 128 + 128)
                nc.sync.dma_start(wqk_t[(qk, hp)][:],
                                  src[:, :, hs].transpose([1, 0, 2]))

            def dma_x(win, eng=None):
                eng = eng or nc.sync
                wsl = slice(T // 4 * win, T // 4 * (win + 1))
                for ck in range(NCK):
                    eng.dma_start(xt_sb[:, ck, wsl], xt[ck, :, wsl])

            # sync queue: weights in need-order; act queue (idle at start):
            # the first two x windows. Later x windows + wp drain on sync
            # behind the weights, finishing long before they are needed.
            dma_w(0, 0)
            dma_w(1, 0)
            dma_x(0, nc.scalar)
            for ck in range(NCK):
                nc.sync.dma_start(wv_sb[:, ck, :], wv[ck])
            nc.sync.dma_start(tri_sb[:], tri[:])
            dma_x(1, nc.scalar)
            for hp in (1, 2, 3):
                dma_w(0, hp)
                dma_w(1, hp)
            dma_x(2)
            dma_x(3)
            for hp in range(HP):
                nc.sync.dma_start(wp_sb[:, hp, :], wp[hp])

            # ---- phase-A work units (emitted inline or as filler) ----
            # NOTE: each unit opens AND closes its psum tile atomically, so
            # arbitrary interleaving of units never splits an open
            # accumulation across other ps_mm.tile() rotations.
            def qk_tile(qk, hp, i):
                dst = q_sb if qk == 0 else k_sb
                ts = slice(512 * i, 512 * i + 512)
                w_h = wqk_t[(qk, hp)]
                p = ps_mm.tile([128, 512], F32, tag="mm", name=f"p{qk}{hp}{i}")
                for ck in range(NCK):
                    nc.tensor.matmul(p[:], w_h[:, ck, :], xt_sb[:, ck, ts],
                                     start=(ck == 0), stop=(ck == NCK - 1),
                                     skip_group_check=True)
                nc.vector.tensor_copy(dst[:, hp, ts], p[:])

            def v_tile(t):
                # token-tile t (128 keys), all head pairs at once
                ts = slice(128 * t, 128 * t + 128)
                p = ps_mm.tile([128, 512], F32, tag="mm", name=f"pv{t}")
                for ck in range(NCK):
                    nc.tensor.matmul(p[:], xt_sb[:, ck, ts], wv_sb[:, ck, :],
                                     start=(ck == 0), stop=(ck == NCK - 1),
                                     skip_group_check=True)
                pv = p[:].rearrange("p (h c) -> p h c", h=HP)
                nc.vector.tensor_copy(v_sb[:, :, t, 0:64], pv[:, :, 0:64])
                nc.vector.tensor_copy(v_sb[:, :, t, 65:129], pv[:, :, 64:128])

            # ---- prelude: enough phase A for (j0, hp0) ----
            for qk in (0, 1):
                qk_tile(qk, 0, 0)
            for t in range(2):
                v_tile(t)

            # ---- filler schedule: (j, hp) -> list of closures ----
            def QK(qk, hp, i):
                return [lambda: qk_tile(qk, hp, i)]

            def V(t):
                return [lambda: v_tile(t)]

            # segment order interleaves j2/j3 so the Act-bound j3 segments
            # can absorb phase-A filler and proj(j2); proj_slots places each
            # projection where its y inputs are 2+ segments old
            seg_order = [(0, 0), (0, 1), (0, 2), (0, 3),
                         (1, 0), (1, 1), (1, 2), (1, 3),
                         (2, 0), (2, 1), (3, 0), (2, 2),
                         (3, 1), (2, 3), (3, 2), (3, 3)]
            proj_slots = {
                (1, 1): {3: (0, (0, 1))},
                (1, 2): {3: (0, (2, 3))},
                (2, 1): {3: (1, (0, 1))},
                (3, 1): {3: (1, (2, 3))},
                (3, 3): {3: (2, (0, 1)), 5: (2, (2, 3))},
            }
            fill = {}
            fill[(0, 0)] = V(2) + V(3) + QK(0, 1, 0) + QK(1, 1, 0)
            fill[(0, 1)] = QK(0, 2, 0) + QK(1, 2, 0)
            fill[(0, 2)] = QK(0, 3, 0) + QK(1, 3, 0)
            fill[(0, 3)] = QK(0, 0, 1) + QK(1, 0, 1) + V(4) + V(5)
            fill[(1, 0)] = V(6) + V(7) + QK(0, 1, 1) + QK(1, 1, 1)
            fill[(1, 1)] = QK(0, 2, 1) + QK(1, 2, 1)
            fill[(1, 2)] = QK(0, 3, 1) + QK(1, 3, 1)
            fill[(1, 3)] = QK(0, 0, 2) + QK(1, 0, 2) + V(8) + V(9)
            fill[(2, 0)] = V(10) + V(11) + QK(0, 1, 2) + QK(1, 1, 2)
            fill[(2, 1)] = QK(0, 0, 3) + QK(1, 0, 3)
            fill[(3, 0)] = V(12) + V(13) + V(14) + V(15) \
                + QK(0, 2, 2) + QK(1, 2, 2)
            fill[(2, 2)] = QK(0, 1, 3) + QK(1, 1, 3)
            fill[(3, 1)] = QK(0, 3, 2) + QK(1, 3, 2)
            fill[(2, 3)] = QK(0, 2, 3) + QK(1, 2, 3)
            fill[(3, 2)] = QK(0, 3, 3) + QK(1, 3, 3)
            fill[(3, 3)] = []

            def emit_proj(j, fs):
                # projection for query blocks fs of block j (y_sb ready)
                for f in fs:
                    t = 4 * j + f
                    ysl = slice(128 * t, 128 * t + 128)
                    ot = outp.tile([128, C], F32, tag="ot")
                    for ch in range(C // 512):
                        po = ps_mm.tile([128, 512], F32, tag="mm",
                                        name=f"po{t}_{ch}")
                        for hp2 in range(HP):
                            nc.tensor.matmul(po[:],
                                             y_t[hp2][:, ysl],
                                             wp_sb[:, hp2, 512 * ch:512 * ch + 512],
                                             start=(hp2 == 0), stop=(hp2 == HP - 1),
                                             skip_group_check=True)
                        nc.vector.tensor_copy(ot[:, 512 * ch:512 * ch + 512], po[:])
                    nc.sync.dma_start(out[128 * t:128 * t + 128, :], ot[:])

            # ---- attention + projection, with filler interleaved ----
            pending = None
            deferred2 = None
            stage3_q = []
            tail_ctx = None
            for j, hp in seg_order:
                ntk = 4 * j + 4
                if True:
                    seg = list(fill[(j, hp)])
                    nseg = len(seg)
                    spread = max(1, (3 * ntk) // 4)

                    def pump(tkb, seg=seg, nseg=nseg, spread=spread):
                        want = (nseg * (tkb + 1) + spread - 1) // spread
                        while seg and (nseg - len(seg)) < min(want, nseg):
                            seg.pop(0)()

                    pyd = ps_y.tile([128, 1024], F32, tag="yd")
                    att_tiles = {}

                    def emit_attv(tkb, att_tiles=att_tiles, pyd=pyd,
                                  hp=hp, ntk=ntk, j=j):
                        r = tkb - 4 * j
                        co = 128 * r if r > 0 else 0
                        att = att_tiles.pop(tkb)
                        st = (tkb == 0)
                        sp = (tkb == ntk - 1)
                        nc.tensor.matmul(pyd[0:65, co:512],
                                         v_sb[:, hp, tkb, 0:65],
                                         att[:, 0, co:512], start=st, stop=sp,
                                         skip_group_check=True)
                        nc.tensor.matmul(pyd[0:65, 512 + co:1024],
                                         v_sb[:, hp, tkb, 65:130],
                                         att[:, 1, co:512], start=st, stop=sp,
                                         skip_group_check=True)

                    for tkb in range(ntk):
                        r = tkb - 4 * j
                        co = 128 * r if r > 0 else 0
                        ks = slice(128 * tkb, 128 * tkb + 128)
                        qs = slice(512 * j + co, 512 * j + 512)
                        pss = ps_s.tile([128, 1024], F32, tag="s")
                        nc.tensor.matmul(pss[:, co:512], k_sb[0:64, hp, ks],
                                         q_sb[0:64, hp, qs],
                                         start=True, stop=True, tile_position=(0, 0),
                                         skip_group_check=True)
                        nc.tensor.matmul(pss[:, 512 + co:1024], k_sb[64:128, hp, ks],
                                         q_sb[64:128, hp, qs],
                                         start=True, stop=True, tile_position=(64, 0),
                                         skip_group_check=True)
                        att = attp.tile([128, 2, 512], BF16, tag="att")
                        att_tiles[tkb] = att
                        pv2 = pss[:].rearrange("p (h t) -> p h t", h=2)
                        nc.scalar.activation(
                            att[:, :, co:512], pv2[:, :, co:512],
                            mybir.ActivationFunctionType.Exp, scale=0.125)
                        if j == NI - 1 and hp == HP - 1 and tkb == ntk - 1:
                            # warm the reciprocal table set now so the
                            # ~2.7us ACT table switch overlaps the last
                            # att*v / proj stream instead of the tail chain
                            junk = nrm.tile([1, 64], F32, tag="junk")
                            nc.scalar.activation(
                                junk[:], ones1[:],
                                mybir.ActivationFunctionType.Ln)
                        if r >= 0:
                            nc.vector.tensor_mul(
                                att[:, :, co:co + 128],
                                att[:, :, co:co + 128],
                                tri_sb[:].rearrange("p (h t) -> p h t", h=2))
                        # defer previous block's tail into this block's score
                        # stream so Act never idles; batch att*v per 2 key
                        # blocks to halve PE mode switches
                        if tkb == 0 and pending is not None:
                            pending()
                            pending = None
                        if tkb == 1 or tkb == 8:
                            for e in list(stage3_q):
                                if e[0] == 0:
                                    e[1]()
                                    stage3_q.remove(e)
                                elif tkb == 1:
                                    e[0] -= 1
                        if tkb == 2 and deferred2 is not None:
                            deferred2()
                            deferred2 = None
                        if tkb % 2 == 1:
                            if tkb >= 3:
                                emit_attv(tkb - 3)
                                emit_attv(tkb - 2)
                            ps = proj_slots.get((j, hp), {}).get(tkb)
                            if ps is not None:
                                emit_proj(ps[0], ps[1])
                        pump(tkb)

                    def emit_tail(emit_attv=emit_attv, pyd=pyd, hp=hp,
                                  ntk=ntk, j=j,
                                  last=(j == NI - 1 and hp == HP - 1)):
                        nonlocal deferred2, tail_ctx
                        emit_attv(ntk - 2)
                        emit_attv(ntk - 1)

                        # ---- normalization, stage 1 ----
                        # yu rows 0:64 = unnormalized y, row 64 = dens;
                        # dens scattered [1,1024]->[128,8] so the reciprocal
                        # runs 128-partition-parallel.  Early rounds route
                        # norm DMAs to the act hwdge queue: the sync queue is
                        # still draining input weights then, and act has
                        # surplus in the PE-bound rounds.
                        dmae = nc.scalar if j <= 1 else nc.sync
                        if last:
                            # dens reciprocal = exp(-ln(d)) on the (idle)
                            # ACT engine: no scatter/gather DMAs in the
                            # tail chain. The DVE copy aligns the den row
                            # to partition 0 (ACT cannot shift partitions).
                            dcopy = nrm.tile([1, 1024], F32, tag="dcopy")
                            nc.vector.tensor_copy(dcopy[:], pyd[64:65, :])
                            lnd = nrm.tile([1, 1024], F32, tag="lnd")
                            nc.scalar.activation(
                                lnd[:], dcopy[:],
                                mybir.ActivationFunctionType.Ln)
                            recb = nrm.tile([1, 1024], BF16, tag="recb")
                            nc.scalar.activation(
                                recb[:], lnd[:],
                                mybir.ActivationFunctionType.Exp, scale=-1.0)
                            yu = nrm.tile([65, 1024], BF16, tag="yu")
                            nc.vector.tensor_copy(yu[:], pyd[0:65, :])
                            tail_ctx = (yu, recb)
                            return
                        yu = nrm.tile([65, 1024], BF16, tag="yu")
                        nc.vector.tensor_copy(yu[:], pyd[0:65, :])
                        den8 = nrm.tile([128, 8], BF16, tag="den8")
                        dmae.dma_start(den8[:], yu[64:65, :])

                        # stage 2 (deferred 2 blocks): recip + gather + bcast
                        def stage2(den8=den8, dmae=dmae):
                            rec8 = nrm.tile([128, 8], F32, tag="rec8")
                            nc.vector.reciprocal(rec8[:], den8[:])
                            rec = nrm.tile([1, 1024], F32, tag="rec")
                            dmae.dma_start(rec[:], rec8[:])
                            dT = nrm.tile([64, 1024], F32, tag="dT")
                            nc.gpsimd.partition_broadcast(dT[:], rec[0:1, :])
                            return dT

                        # stage 3 (deferred two segments): the y scaling,
                        # past the gpsimd latency so the muls never
                        # head-block the DVE queue
                        def stage3(yu=yu, hp=hp, j=j, dmae=dmae):
                            dT = stage3.dT
                            tqs = slice(512 * j, 512 * j + 512)
                            nc.vector.tensor_mul(y_t[hp][0:64, tqs],
                                                 yu[0:64, 0:512], dT[:, 0:512])
                            yu2 = nrm.tile([64, 512], BF16, tag="yu2")
                            nc.vector.tensor_mul(yu2[:], yu[0:64, 512:1024],
                                                 dT[:, 512:1024])
                            dmae.dma_start(y_t[hp][64:128, tqs], yu2[:])

                        def run_stage2(stage2=stage2, stage3=stage3):
                            stage3.dT = stage2()

                        deferred2 = run_stage2
                        # deeper deferral for the short j0 segments: their
                        # norm chains are still DMA-latency-bound at startup
                        stage3_q.append([2 if (j == 0 and hp < 3) else 1,
                                         stage3])

                    pending = emit_tail

            # ---- tail: last normalization via PE broadcast, with proj(j3)
            # partials for hp0..2 overlapping the whole chain so the PE
            # stays warm and only the hp3 contributions remain at the end
            for e in stage3_q:
                e[1]()
            stage3_q = []
            pending()
            yu, recb = tail_ctx

            def proj_part(po, t, ch, hps, start, stop):
                ysl = slice(128 * t, 128 * t + 128)
                for hp2 in hps:
                    nc.tensor.matmul(po[:, 512 * ch:512 * ch + 512],
                                     y_t[hp2][:, ysl],
                                     wp_sb[:, hp2, 512 * ch:512 * ch + 512],
                                     start=(start and hp2 == hps[0]),
                                     stop=(stop and hp2 == hps[-1]),
                                     skip_group_check=True)

            tb = 4 * (NI - 1)
            po_t = {}
            ysl = slice(128 * tb, 128 * tb + 128)
            for ch in (0, 1):
                po = ps_mm.tile([128, 512], F32, tag="mm", name=f"pot0{ch}")
                for hp2 in (0, 1, 2):
                    nc.tensor.matmul(po[:], y_t[hp2][:, ysl],
                                     wp_sb[:, hp2, 512 * ch:512 * ch + 512],
                                     start=(hp2 == 0), stop=False,
                                     skip_group_check=True)
                po_t[(0, ch)] = po
            po_t[1] = ps_s.tile([128, 1024], F32, tag="s", name="pot1")
            po_t[2] = ps_s.tile([128, 1024], F32, tag="s", name="pot2")
            # t1/t2 hp0..2 partials (PE work during the normalization chain)
            for t in (1, 2):
                proj_part(po_t[t], tb + t, 0, (0, 1, 2), True, False)
                proj_part(po_t[t], tb + t, 1, (0, 1, 2), True, False)
            pbc = ps_y.tile([128, 1024], F32, tag="yd", name="pbc")
            nc.tensor.matmul(pbc[0:64, 0:512], ones1[0:1, :], recb[0:1, 0:512],
                             start=True, stop=True, skip_group_check=True)
            nc.tensor.matmul(pbc[0:64, 512:1024], ones1[0:1, :],
                             recb[0:1, 512:1024],
                             start=True, stop=True, skip_group_check=True)
            tqs = slice(512 * (NI - 1), 512 * NI)
            nc.vector.tensor_mul(y_t[HP - 1][0:64, tqs],
                                 yu[0:64, 0:512], pbc[0:64, 0:512])
            yu2 = nrm.tile([64, 512], BF16, tag="yu2")
            nc.vector.tensor_mul(yu2[:], yu[0:64, 512:1024],
                                 pbc[0:64, 512:1024])
            nc.sync.dma_start(y_t[HP - 1][64:128, tqs], yu2[:])

            # hp3 contributions + copy-out
            ot0 = outp.tile([128, C], F32, tag="ot")
            ysl = slice(128 * tb, 128 * tb + 128)
            for ch in (0, 1):
                nc.tensor.matmul(po_t[(0, ch)][:], y_t[HP - 1][:, ysl],
                                 wp_sb[:, HP - 1, 512 * ch:512 * ch + 512],
                                 start=False, stop=True, skip_group_check=True)
                nc.vector.tensor_copy(ot0[:, 512 * ch:512 * ch + 512],
                                      po_t[(0, ch)][:])
            nc.sync.dma_start(out[128 * tb:128 * tb + 128, :], ot0[:])
            for t in (1, 2):
                proj_part(po_t[t], tb + t, 0, (3,), False, True)
                proj_part(po_t[t], tb + t, 1, (3,), False, True)
                ot = outp.tile([128, C], F32, tag="ot")
                nc.vector.tensor_copy(ot[:, 0:512], po_t[t][:, 0:512])
                nc.vector.tensor_copy(ot[:, 512:1024], po_t[t][:, 512:1024])
                nc.sync.dma_start(out[128 * (tb + t):128 * (tb + t) + 128, :],
                                  ot[:])
            emit_proj(NI - 1, (3,))

    nc.compile()
    return nc


def make_inputs(x_b, w_qkv, w_proj, g, HL=8):
    """Host-side prep of one core's input map.

    x_b: [T, C] fp32 (one batch), g: head-group index (0 or 1).
    """
    import ml_dtypes
    BF = ml_dtypes.bfloat16
    T, C = x_b.shape
    D = 64
    NCK = C // 128
    HP = HL // 2
    h0 = g * HL * D
    xt = np.ascontiguousarray(x_b.T.reshape(NCK, 128, T)).astype(BF)
    wq = np.ascontiguousarray(
        w_qkv[:, h0:h0 + HL * D].reshape(NCK, 128, HL * D)).astype(BF)
    wk = np.ascontiguousarray(
        w_qkv[:, C + h0:C + h0 + HL * D].reshape(NCK, 128, HL * D)).astype(BF)
    wv = np.ascontiguousarray(
        w_qkv[:, 2 * C + h0:2 * C + h0 + HL * D].reshape(NCK, 128, HL * D)).astype(BF)
    wp = np.ascontiguousarray(
        w_proj[h0:h0 + HL * D, :].reshape(HP, 128, C)).astype(BF)
    t1 = np.triu(np.ones((128, 128), dtype=np.float32))
    tri = np.concatenate([t1, t1], axis=1).astype(BF)
    return {"xt": xt, "wq": wq, "wk": wk, "wv": wv, "wp": wp, "tri": tri}


_NC_CACHE = {}


def kernel(x, w_qkv, w_proj):
    import numpy as np
    from concourse.bass_utils import run_bass_kernel_spmd

    x = np.ascontiguousarray(np.asarray(x, dtype=np.float32))
    w_qkv = np.ascontiguousarray(np.asarray(w_qkv, dtype=np.float32))
    w_proj = np.ascontiguousarray(np.asarray(w_proj, dtype=np.float32))
    B, T, C = x.shape

    key = (T, C)
    if key not in _NC_CACHE:
        _NC_CACHE[key] = build(T=T, HL=8, C=C)
    nc = _NC_CACHE[key]

    in_maps = [make_inputs(x[c // 2], w_qkv, w_proj, c % 2, HL=8) for c in range(8)]
    res = run_bass_kernel_spmd(nc, in_maps, core_ids=list(range(8)), trace=False)

    out = np.zeros((B, T, C), dtype=np.float32)
    for c in range(8):
        out[c // 2] += res.results[c]["out"]
    return out
